# revision 58
# baseline (speedup 1.0000x reference)
"""nn_CrossAttention kernel v11 for 8x TRN2 NeuronCores. ~363-373us HW
(v2 baseline: 746us). Core c = (batch b = c//2, head-group hg = c%2).

Design, distilled from the HAM/throttle investigation:
 - The PE must be the pacer everywhere and must never idle a full
   ~3.4us HAM window, or it gets clock-gated to 1.2 GHz (and a
   saturated-cold PE never recovers). Everything below serves that.
 - x and W ship from the host as bf16 (halves DMA, [128,1024] x tiles
   = 2KB DMA lines); projections, QK and AV all run bf16 at 1 col/cyc.
 - exp over sc bank-PAIRS [128,2,512]: one 1024-wide op per i-slot.
   ACT takes 11 pairs/phase (exact exp), DVE 5 via a Schraudolph-style
   bit-trick (int16 bits = round(score*A+B) = bf16 exp) -> rel err
   ~1.1e-2 vs the 2e-2 gate.
 - Phase boundary: av PSUM banks are freed by quick DVE copies (DVE's
   last trick pair is slot 14, so it's idle at the boundary); the
   denominator reciprocal runs on a [64,8] DRAM-bounce spread (6
   cyc/elem DVE reciprocal costs 8 elems/lane, not 512); the
   normalize-multiply runs on the otherwise-idle Pool engine
   (all-SBUF); norm head/tail split so nothing blocks the DVE FIFO.
 - Prologue: kT t0/qT t0 from single-t tiles for the shortest path to
   the first QK (~37us); phase-0 QK blocks interleave with the
   remaining kT passes; v-projection interleaves with phase 1.
 - PSUM: vp 2 + sc 2x2 + av 2 = 8 banks.
 - Per-dd W tiles (first projection MM waits one 128KB DMA, not 1MB);
   AV MMs grouped by head across i-slot pairs (same-bank b2b
   accumulation).
 - NOTE: the device transiently enters a ~20% all-engine slow state
   (this identical kernel measured 363/431/365us across runs); judge
   any change only against a same-window re-run of the old binary.
"""

import json
import numpy as np

B, S, D, NH, HD = 4, 2048, 1024, 16, 64
CPC = 512          # cols per core = 8 heads * 64
NCORES = 8
NDT = D // 128     # 8 d-tiles
NP = CPC // 128    # 4 c-tiles (head pairs)
NSK = S // 128     # 16 sk-tiles
NJ = S // 512      # 4 sq chunks
SQC = 512          # sq chunk size
NPH = NP * NJ      # 16 phases

# bit-trick exp constants: bf16 bits = round(score * EXPA + EXPB)
# exp(s*0.125) = 2^(s*0.125*log2 e); bf16 bits = exp_field*128 + mantissa
EXPA = 0.125 * 1.4426950408889634 * 128.0   # 23.0831...
EXPB = 16256.0 - 7.5                        # 127*128 - sigma (sigma tuned)

# i-slot -> engine assignment: DVE trick pairs (5 of 16 per phase)
DVE_PAIRS_I = frozenset((2, 5, 8, 11, 14))


# ---------------------------------------------------------------- drain fix
def _fix_module_json(bj: bytes) -> bytes:
    """This walrus build accepts at most ONE sync wait/update on CTRL-lowered
    instructions (Drain). Move extras onto EventSemaphore instructions."""
    d = json.loads(bj)
    counter = [0]

    def fix_block(b):
        out = []
        for inst in b.get("instructions", []):
            si = inst.get("sync_info") or {}
            ow = si.get("on_wait") or []
            ou = si.get("on_update") or []
            if (inst.get("opcode") not in
                    ("EventSemaphore", "Call", "RegisterMove",
                     "UnconditionalBranch", "ISA", "Drain") and len(ow) > 1):
                for w in ow[1:]:
                    counter[0] += 1
                    out.append({
                        "debug": inst.get("debug", 0),
                        "engine": inst["engine"],
                        "ins": [], "outs": [],
                        "name": f"synthmmw-{counter[0]}",
                        "opcode": "EventSemaphore",
                        "sync_info": {"on_update": [], "on_wait": [w]},
                    })
                inst["sync_info"] = {"on_update": ou, "on_wait": ow[:1]}
                out.append(inst)
                continue
            if inst.get("opcode") == "Drain" and (len(ow) > 1 or len(ou) > 1):
                for w in ow[1:]:
                    counter[0] += 1
                    out.append({
                        "debug": inst.get("debug", 0),
                        "engine": inst["engine"],
                        "ins": [], "outs": [],
                        "name": f"synthwait-{counter[0]}",
                        "opcode": "EventSemaphore",
                        "sync_info": {"on_update": [], "on_wait": [w]},
                    })
                inst["sync_info"] = {"on_update": ou[:1], "on_wait": ow[:1]}
                out.append(inst)
                for u in ou[1:]:
                    counter[0] += 1
                    out.append({
                        "debug": inst.get("debug", 0),
                        "engine": inst["engine"],
                        "ins": [], "outs": [],
                        "name": f"synthupd-{counter[0]}",
                        "opcode": "EventSemaphore",
                        "sync_info": {"on_update": [u], "on_wait": []},
                    })
            else:
                out.append(inst)
        b["instructions"] = out
        for sb in b.get("blocks", []):
            fix_block(sb)

    for fn in d.get("functions", []):
        for blk in fn.get("blocks", []):
            fix_block(blk)
    return json.dumps(d).encode()


def _install_drainfix():
    import concourse.bass as bass
    if getattr(bass.Bass, "_drainfix_installed", False):
        return
    orig = bass.Bass.to_json_bytes

    def patched(self):
        return _fix_module_json(orig(self))

    bass.Bass.to_json_bytes = patched
    bass.Bass._drainfix_installed = True


# ---------------------------------------------------------------- program
def _build_nc(reps=1):
    import concourse.bass as bass
    import concourse.mybir as mybir
    from concourse.tile import TileContext
    from contextlib import ExitStack

    f32 = mybir.dt.float32
    f32r = mybir.dt.float32r
    bf16 = mybir.dt.bfloat16
    i16 = mybir.dt.int16
    EXP = mybir.ActivationFunctionType.Exp
    MUL = mybir.AluOpType.mult
    ADD = mybir.AluOpType.add

    nc = bass.Bass("TRN2", num_devices=NCORES)

    xqT = nc.dram_tensor("xqT", [D, S], bf16, kind="ExternalInput")
    xkT = nc.dram_tensor("xkT", [D, S], bf16, kind="ExternalInput")
    xvT = nc.dram_tensor("xvT", [D, S], bf16, kind="ExternalInput")
    wq = nc.dram_tensor("wq", [D, CPC], bf16, kind="ExternalInput")
    wk = nc.dram_tensor("wk", [D, CPC], bf16, kind="ExternalInput")
    wv = nc.dram_tensor("wv", [D, CPC], bf16, kind="ExternalInput")
    bqd = nc.dram_tensor("bq", [CPC], f32, kind="ExternalInput")
    bkd = nc.dram_tensor("bk", [CPC], f32, kind="ExternalInput")
    bvd = nc.dram_tensor("bv", [CPC], f32, kind="ExternalInput")
    outd = nc.dram_tensor("out", [CPC, S], f32, kind="ExternalOutput")
    # scratch for the denominator-reciprocal broadcast bounce (stride-0
    # partition reads are only legal from DRAM)
    recscr = nc.dram_tensor("recscr", [2 * NPH, SQC], f32, kind="Internal")

    with ExitStack() as ctx:
        ctx.enter_context(nc.allow_low_precision(
            reason="qk in bf16 + bit-trick exp; matmul accumulates f32; "
                   "rel-err gate 2e-2"))
        tc = ctx.enter_context(TileContext(nc))
        sb = ctx.enter_context(tc.tile_pool(name="sb", bufs=1))
        ps = ctx.enter_context(tc.tile_pool(name="ps", bufs=1, space="PSUM"))

        # ---- resident weights / constants (wk + xk first: critical) ----
        # per-dd W tiles: the first projection MM waits only on dd=0's DMA
        wk_sb = [sb.tile([128, CPC], bf16, name=f"wk_sb{dd}")
                 for dd in range(NDT)]
        for dd in range(NDT):
            nc.sync.dma_start(out=wk_sb[dd],
                              in_=wk[dd * 128:(dd + 1) * 128, :])
        bk_sb = sb.tile([128, NP], f32, name="bk_sb")
        nc.sync.dma_start(out=bk_sb, in_=bkd.rearrange("(p c) -> c p", p=NP))
        bq_sb = sb.tile([128, NP], f32, name="bq_sb")
        nc.sync.dma_start(out=bq_sb, in_=bqd.rearrange("(p c) -> c p", p=NP))

        # persistent activation tiles
        qT = [sb.tile([128, S], bf16, name=f"qT{p}") for p in range(NP)]
        kT = [sb.tile([128, S], bf16, name=f"kT{p}") for p in range(NP)]
        v_aug = sb.tile([128, NSK, 8, 65], bf16, name="v_aug")

        # ---- helpers ----
        def load_x2(x_dram, u):
            """8 [128, 1024] bf16 tiles covering t-chunks 2u, 2u+1 (2KB
            per-partition DMA lines)."""
            xts = []
            for dd in range(NDT):
                x_t = sb.tile([128, 2 * SQC], bf16, tag="xs", bufs=10,
                              name=f"x_{u}_{dd}")
                nc.sync.dma_start(
                    out=x_t,
                    in_=x_dram[dd * 128:(dd + 1) * 128,
                               u * 2 * SQC:(u + 1) * 2 * SQC])
                xts.append(x_t)
            return xts

        def load_x1(x_dram, t):
            """8 single-t [128, 512] bf16 tiles (1KB lines; used for the qT
            passes so the critical path to the first QK is shorter)."""
            xts = []
            for dd in range(NDT):
                x_t = sb.tile([128, SQC], bf16, tag="xs1", bufs=16,
                              name=f"x1_{t}_{dd}")
                nc.sync.dma_start(
                    out=x_t,
                    in_=x_dram[dd * 128:(dd + 1) * 128,
                               t * SQC:(t + 1) * SQC])
                xts.append(x_t)
            return xts

        def emit_proj(t, plist, w_sb, bias_sb, dst, xts, single=False):
            """dst[p][:, t*512:(t+1)*512] = (x @ W + b).T chunk (bf16).
            xts are the [128, 1024] pair tiles for u = t//2 (or single-t
            [128, 512] tiles if single)."""
            xo = 0 if single else (t % 2) * SQC
            for p in plist:
                pr = ps.tile([128, SQC], f32, tag="vp", bufs=2,
                             name=f"prj_{t}_{p}")
                for dd in range(NDT):
                    nc.tensor.matmul(
                        pr[:, :],
                        w_sb[dd][:, p * 128:(p + 1) * 128],
                        xts[dd][:, xo:xo + SQC],
                        start=(dd == 0), stop=(dd == NDT - 1))
                nc.vector.tensor_scalar_add(
                    dst[p][:, t * SQC:(t + 1) * SQC], pr[:, :],
                    bias_sb[:, p:p + 1])

        xv_group = {}  # g -> list of 8 [128, 512] bf16 tiles (tt 4g..4g+3)

        def load_xv(g):
            xvt = []
            for dd in range(NDT):
                xv_t = sb.tile([128, 4 * 128], bf16, tag="xv", bufs=12,
                               name=f"xv_{g}_{dd}")
                nc.sync.dma_start(
                    out=xv_t,
                    in_=xvT[dd * 128:(dd + 1) * 128,
                            g * 512:(g + 1) * 512])
                xvt.append(xv_t)
            xv_group[g] = xvt

        def emit_proj_v(tt):
            """v_aug[:, tt, h, 0:64] = (xv @ Wv + bv) rows tt*128.., bf16."""
            xvt = xv_group[tt // 4]
            co = (tt % 4) * 128
            pv = ps.tile([128, CPC], f32, tag="vp", bufs=2, name=f"pv_{tt}")
            for dd in range(NDT):
                nc.tensor.matmul(
                    pv[:, :], xvt[dd][:, co:co + 128], wv_sb[dd][:, :],
                    start=(dd == 0), stop=(dd == NDT - 1))
            nc.vector.tensor_add(
                v_aug[:, tt, :, 0:64],
                pv.rearrange("c (h d) -> c h d", h=8),
                bv_bc.rearrange("c (h d) -> c h d", h=8))
            if tt % 4 == 3:
                xv_group.pop(tt // 4)

        alpha = {}     # (c, i) -> [128, 2, 512] bf16 pair tile
        av_tiles = {}  # c -> {h: psum tile [65, 512]}

        def emit_qk_i(c, i):
            p, j = divmod(c, NJ)
            scp = ps.tile([128, 2, SQC], f32, tag="sc", bufs=2,
                          name=f"sc_{c}_{i}")
            for h in range(2):
                nc.tensor.matmul(
                    scp[:, h, :],
                    kT[p][h * 64:(h + 1) * 64, i * 128:(i + 1) * 128],
                    qT[p][h * 64:(h + 1) * 64, j * SQC:(j + 1) * SQC],
                    start=True, stop=True)
            a_p = sb.tile([128, 2, SQC], bf16, tag="alpha", bufs=18,
                          name=f"al_{c}_{i}")
            if i in DVE_PAIRS_I:
                nc.vector.tensor_scalar(
                    a_p.bitcast(i16), scp[:, :, :], EXPA, EXPB, MUL, ADD)
            else:
                nc.scalar.activation(a_p[:, :, :], scp[:, :, :], EXP,
                                     scale=0.125)
            alpha[(c, i)] = a_p

        def emit_av_i(c, i):
            p, j = divmod(c, NJ)
            if i == 0:
                av_tiles[c] = {
                    h: ps.tile([128, SQC], f32,
                               tag=("av" if c % 2 == 0 else "vp"), bufs=2,
                               name=f"av_{c}_{h}")
                    for h in range(2)}
            a_p = alpha[(c, i)]
            for h in range(2):
                nc.tensor.matmul(
                    av_tiles[c][h][0:65, :],
                    v_aug[:, i, 2 * p + h, :],
                    a_p[:, h, :],
                    start=(i == 0), stop=(i == NSK - 1))
            alpha.pop((c, i))

        norm_state = {}  # c -> (avs, recB)

        def emit_norm_head(c):
            """Free the av PSUM banks FAST: both copies on DVE (whose last
            trick pair is slot 14, so it's free at the boundary). Then the
            denominator bounce: raw row -> DRAM -> [64,8] spread so the
            6-cycles/elem DVE reciprocal costs 8 elems/lane -> DRAM ->
            [64,512] broadcast."""
            avt = av_tiles.pop(c)
            avs, recB = {}, {}
            for h in range(2):
                avs[h] = sb.tile([65, SQC], f32, tag="avs", bufs=4,
                                 name=f"avs_{c}_{h}")
                nc.vector.tensor_copy(avs[h][:, :], avt[h][0:65, :])
            for h in range(2):
                slot = 2 * c + h
                nc.sync.dma_start(out=recscr[slot:slot + 1, :],
                                  in_=avs[h][64:65, :])
                _r = recscr[slot:slot + 1, :]
                den8 = sb.tile([64, 8], f32, tag="den8", bufs=4,
                               name=f"den8_{c}_{h}")
                nc.sync.dma_start(
                    out=den8,
                    in_=bass.AP(tensor=_r.tensor, offset=_r.offset,
                                ap=[[8, 64], [1, 8]]))
                rec8 = sb.tile([64, 8], f32, tag="rec8", bufs=4,
                               name=f"rec8_{c}_{h}")
                nc.vector.reciprocal(rec8[:, :], den8[:, :])
                nc.sync.dma_start(
                    out=bass.AP(tensor=_r.tensor, offset=_r.offset,
                                ap=[[8, 64], [1, 8]]),
                    in_=rec8[:, :])
                recB[h] = sb.tile([64, SQC], f32, tag="recB", bufs=4,
                                  name=f"recB_{c}_{h}")
                nc.sync.dma_start(
                    out=recB[h],
                    in_=bass.AP(tensor=_r.tensor, offset=_r.offset,
                                ap=[[0, 64]] + list(_r.ap)[1:]))
            norm_state[c] = (avs, recB)

        def _emit_norm_last(c, h):
            """Latency-optimized norm for the final chunk: direct DVE
            reciprocal of the row (no [64,8] spread), ONE DRAM bounce for
            the broadcast, DVE multiply."""
            p, j = divmod(c, NJ)
            avt = av_tiles[c]
            r0 = (2 * p + h) * 64
            slot = 2 * c + h
            avs = sb.tile([65, SQC], f32, tag="avs", bufs=4,
                          name=f"avsL_{h}")
            nc.vector.tensor_copy(avs[:, :], avt[h][0:65, :])
            rec = sb.tile([1, SQC], f32, tag="recL", bufs=2,
                          name=f"recL_{h}")
            nc.vector.reciprocal(rec[:, :], avs[64:65, :])
            nc.sync.dma_start(out=recscr[slot:slot + 1, :], in_=rec[:, :])
            _r = recscr[slot:slot + 1, :]
            recB = sb.tile([64, SQC], f32, tag="recB", bufs=4,
                           name=f"recBL_{h}")
            nc.sync.dma_start(
                out=recB,
                in_=bass.AP(tensor=_r.tensor, offset=_r.offset,
                            ap=[[0, 64]] + list(_r.ap)[1:]))
            cx = sb.tile([64, SQC], f32, tag="cx", bufs=4,
                         name=f"cxL_{h}")
            nc.vector.tensor_mul(cx[:, :], avs[0:64, :], recB[:, :])
            nc.sync.dma_start(
                out=outd[r0:r0 + 64, j * SQC:(j + 1) * SQC],
                in_=cx[:, :])

        def emit_norm_tail(c):
            """Normalize-multiply on the idle Pool engine (all-SBUF), DMA
            out. Emitted mid-next-phase so the recB bounce has landed."""
            p, j = divmod(c, NJ)
            avs, recB = norm_state.pop(c)
            for h in range(2):
                r0 = (2 * p + h) * 64
                cx = sb.tile([64, SQC], f32, tag="cx", bufs=4,
                             name=f"cx_{c}_{h}")
                nc.gpsimd.tensor_mul(cx[:, :], avs[h][0:64, :], recB[h][:, :])
                nc.sync.dma_start(
                    out=outd[r0:r0 + 64, j * SQC:(j + 1) * SQC],
                    in_=cx[:, :])

        # late-loaded residents (emitted after critical-path DMAs above,
        # but data only needed from mid-prologue onwards)
        wq_sb = [sb.tile([128, CPC], bf16, name=f"wq_sb{dd}")
                 for dd in range(NDT)]
        wv_sb = [sb.tile([128, CPC], bf16, name=f"wv_sb{dd}")
                 for dd in range(NDT)]
        bv_bc = sb.tile([128, CPC], f32, name="bv_bc")

        def _emit_wq():
            for dd in range(NDT):
                nc.sync.dma_start(
                    out=wq_sb[dd],
                    in_=wq[dd * 128:(dd + 1) * 128, :])

        def _emit_wv():
            for dd in range(NDT):
                nc.sync.dma_start(
                    out=wv_sb[dd],
                    in_=wv[dd * 128:(dd + 1) * 128, :])
            _bva = bvd[:]
            nc.sync.dma_start(
                out=bv_bc,
                in_=bass.AP(tensor=_bva.tensor, offset=_bva.offset,
                            ap=[[0, 128]] + list(_bva.ap)))
            nc.gpsimd.memset(v_aug[:, :, :, 64:65], 1.0)

        # ---- emission schedule ----
        def _emit_all():
            # prologue (x/W in bf16, [128,1024] x pair-tiles): kT t0/t1 ->
            # qT t0/t1 -> phase-0 QK 0..7 -> kT t2/t3 -> QK 8..15 ->
            # qT t2/t3 -> wv. wq DMA right after xk u0 so the first qT
            # chain isn't blocked.
            xkt0 = load_x1(xkT, 0)
            _emit_wq()
            emit_proj(0, [0, 1, 2, 3], wk_sb, bk_sb, kT, xkt0, single=True)
            xq0 = load_x1(xqT, 0)
            emit_proj(0, [0, 1, 2, 3], wq_sb, bq_sb, qT, xq0, single=True)
            for i in range(4):
                emit_qk_i(0, i)
            xkt1 = load_x1(xkT, 1)
            emit_proj(1, [0, 1, 2, 3], wk_sb, bk_sb, kT, xkt1, single=True)
            xk1 = load_x2(xkT, 1)
            emit_proj(2, [0, 1, 2, 3], wk_sb, bk_sb, kT, xk1)
            for i in range(4, 8):
                emit_qk_i(0, i)
            emit_proj(3, [0, 1, 2, 3], wk_sb, bk_sb, kT, xk1)
            for i in range(8, 12):
                emit_qk_i(0, i)
            xq1 = load_x1(xqT, 1)
            emit_proj(1, [0, 1, 2, 3], wq_sb, bq_sb, qT, xq1, single=True)
            for i in range(12, 16):
                emit_qk_i(0, i)
            xq2 = load_x1(xqT, 2)
            emit_proj(2, [0, 1, 2, 3], wq_sb, bq_sb, qT, xq2, single=True)
            xq3 = load_x1(xqT, 3)
            emit_proj(3, [0, 1, 2, 3], wq_sb, bq_sb, qT, xq3, single=True)
            _emit_wv()

            # phase 1: v-projection (xv DMA paced) interleaved with QK(1)
            # and AV(0) (AV(0,i) needs v_aug[:, i] just computed).
            for i in range(NSK):
                if i == 0:
                    load_xv(0)
                if i % 4 == 1 and i // 4 + 1 < 4:
                    load_xv(i // 4 + 1)
                emit_proj_v(i)
                emit_qk_i(1, i)
                emit_av_i(0, i)
            emit_norm_head(0)

            # phases 2..15: pure QK/AV; norm tail of phase c-2 mid-phase
            # (after its recB bounce has landed), norm head of c-1 at end.
            # AV MMs grouped by head across i-slot PAIRS so consecutive AV
            # MMs accumulate into the SAME PSUM bank (drain overlaps fill).
            def emit_av_pair(c, ii):
                p, j = divmod(c, NJ)
                if ii == 0:
                    av_tiles[c] = {
                        h: ps.tile([128, SQC], f32,
                                   tag=("av" if c % 2 == 0 else "vp"), bufs=2,
                                   name=f"av_{c}_{h}")
                        for h in range(2)}
                for h in range(2):
                    for i2 in (ii, ii + 1):
                        nc.tensor.matmul(
                            av_tiles[c][h][0:65, :],
                            v_aug[:, i2, 2 * p + h, :],
                            alpha[(c, i2)][:, h, :],
                            start=(i2 == 0), stop=(i2 == NSK - 1))
                alpha.pop((c, ii))
                alpha.pop((c, ii + 1))

            for c in range(2, NPH):
                for ii in range(0, NSK, 2):
                    emit_qk_i(c, ii)
                    emit_qk_i(c, ii + 1)
                    emit_av_pair(c - 1, ii)
                    if ii == 6:
                        emit_norm_tail(c - 2)
                emit_norm_head(c - 1)

            # epilogue: run h0's AV chain first so norm(15) h0 (copy +
            # direct reciprocal + single bounce + DVE mul) overlaps h1's
            # chain; latency-optimized norm path for the final chunk.
            c = NPH - 1
            p, j = divmod(c, NJ)
            av_tiles[c] = {
                h: ps.tile([128, SQC], f32, tag="vp", bufs=2,
                           name=f"av_{c}_{h}")
                for h in range(2)}
            for h in range(2):
                for i in range(NSK):
                    nc.tensor.matmul(
                        av_tiles[c][h][0:65, :],
                        v_aug[:, i, 2 * p + h, :],
                        alpha[(c, i)][:, h, :],
                        start=(i == 0), stop=(i == NSK - 1))
                if h == 0:
                    emit_norm_tail(NPH - 2)
                    _emit_norm_last(c, 0)
            for i in range(NSK):
                alpha.pop((c, i))
            _emit_norm_last(c, 1)

        for _rep in range(reps):
            _emit_all()

    return nc


_NC_BY_REPS = {}


def _get_nc(reps=1):
    if reps not in _NC_BY_REPS:
        _install_drainfix()
        _NC_BY_REPS[reps] = _build_nc(reps)
    return _NC_BY_REPS[reps]


# ---------------------------------------------------------------- entry
def build_in_maps(inputs):
    import ml_dtypes

    bf16 = ml_dtypes.bfloat16
    query = np.asarray(inputs["query"], np.float32)
    key_in = np.asarray(inputs["key_in"], np.float32)
    value = np.asarray(inputs["value"], np.float32)
    Wq = np.asarray(inputs["Wq"], np.float32)
    Wk = np.asarray(inputs["Wk"], np.float32)
    Wv = np.asarray(inputs["Wv"], np.float32)
    bq = np.asarray(inputs["bq"], np.float32)
    bk = np.asarray(inputs["bk"], np.float32)
    bv = np.asarray(inputs["bv"], np.float32)

    in_maps = []
    for c in range(NCORES):
        b, hg = divmod(c, 2)
        cols = slice(hg * CPC, (hg + 1) * CPC)
        in_maps.append({
            "xqT": np.ascontiguousarray(query[b].T.astype(bf16)),
            "xkT": np.ascontiguousarray(key_in[b].T.astype(bf16)),
            "xvT": np.ascontiguousarray(value[b].T.astype(bf16)),
            "wq": np.ascontiguousarray(Wq[:, cols].astype(bf16)),
            "wk": np.ascontiguousarray(Wk[:, cols].astype(bf16)),
            "wv": np.ascontiguousarray(Wv[:, cols].astype(bf16)),
            "bq": np.ascontiguousarray(bq[cols]),
            "bk": np.ascontiguousarray(bk[cols]),
            "bv": np.ascontiguousarray(bv[cols]),
        })
    return in_maps


def kernel(query, key_in, value, Wq, bq, Wk, bk, Wv, bv):
    from concourse.bass_utils import run_bass_kernel_spmd

    nc = _get_nc()
    in_maps = build_in_maps({
        "query": query, "key_in": key_in, "value": value,
        "Wq": Wq, "bq": bq, "Wk": Wk, "bk": bk, "Wv": Wv, "bv": bv,
    })

    res = run_bass_kernel_spmd(nc, in_maps, core_ids=list(range(NCORES)))

    out = np.empty((B, S, D), np.float32)
    for c in range(NCORES):
        b, hg = divmod(c, 2)
        out[b, :, hg * CPC:(hg + 1) * CPC] = res.results[c]["out"].T
    return out


# revision 59
# speedup vs baseline: 1.0096x; 1.0096x over previous
"""nn_CrossAttention kernel v11 for 8x TRN2 NeuronCores. ~363-373us HW
(v2 baseline: 746us). Core c = (batch b = c//2, head-group hg = c%2).

Design, distilled from the HAM/throttle investigation:
 - The PE must be the pacer everywhere and must never idle a full
   ~3.4us HAM window, or it gets clock-gated to 1.2 GHz (and a
   saturated-cold PE never recovers). Everything below serves that.
 - x and W ship from the host as bf16 (halves DMA, [128,1024] x tiles
   = 2KB DMA lines); projections, QK and AV all run bf16 at 1 col/cyc.
 - exp over sc bank-PAIRS [128,2,512]: one 1024-wide op per i-slot.
   ACT takes 11 pairs/phase (exact exp), DVE 5 via a Schraudolph-style
   bit-trick (int16 bits = round(score*A+B) = bf16 exp) -> rel err
   ~1.1e-2 vs the 2e-2 gate.
 - Phase boundary: av PSUM banks are freed by quick DVE copies (DVE's
   last trick pair is slot 14, so it's idle at the boundary); the
   denominator reciprocal runs on a [64,8] DRAM-bounce spread (6
   cyc/elem DVE reciprocal costs 8 elems/lane, not 512); the
   normalize-multiply runs on the otherwise-idle Pool engine
   (all-SBUF); norm head/tail split so nothing blocks the DVE FIFO.
 - Prologue: kT t0/qT t0 from single-t tiles for the shortest path to
   the first QK (~37us); phase-0 QK blocks interleave with the
   remaining kT passes; v-projection interleaves with phase 1.
 - PSUM: vp 2 + sc 2x2 + av 2 = 8 banks.
 - Per-dd W tiles (first projection MM waits one 128KB DMA, not 1MB);
   AV MMs grouped by head across i-slot pairs (same-bank b2b
   accumulation).
 - NOTE: the device transiently enters a ~20% all-engine slow state
   (this identical kernel measured 363/431/365us across runs); judge
   any change only against a same-window re-run of the old binary.
"""

import json
import numpy as np

B, S, D, NH, HD = 4, 2048, 1024, 16, 64
CPC = 512          # cols per core = 8 heads * 64
NCORES = 8
NDT = D // 128     # 8 d-tiles
NP = CPC // 128    # 4 c-tiles (head pairs)
NSK = S // 128     # 16 sk-tiles
NJ = S // 512      # 4 sq chunks
SQC = 512          # sq chunk size
NPH = NP * NJ      # 16 phases

# bit-trick exp constants: bf16 bits = round(score * EXPA + EXPB)
# exp(s*0.125) = 2^(s*0.125*log2 e); bf16 bits = exp_field*128 + mantissa
EXPA = 0.125 * 1.4426950408889634 * 128.0   # 23.0831...
EXPB = 16256.0 - 7.5                        # 127*128 - sigma (sigma tuned)

# i-slot -> engine assignment: DVE trick pairs (5 of 16 per phase)
DVE_PAIRS_I = frozenset((2, 5, 8, 11, 14))


# ---------------------------------------------------------------- drain fix
def _fix_module_json(bj: bytes) -> bytes:
    """This walrus build accepts at most ONE sync wait/update on CTRL-lowered
    instructions (Drain). Move extras onto EventSemaphore instructions."""
    d = json.loads(bj)
    counter = [0]

    def fix_block(b):
        out = []
        for inst in b.get("instructions", []):
            si = inst.get("sync_info") or {}
            ow = si.get("on_wait") or []
            ou = si.get("on_update") or []
            if (inst.get("opcode") not in
                    ("EventSemaphore", "Call", "RegisterMove",
                     "UnconditionalBranch", "ISA", "Drain") and len(ow) > 1):
                for w in ow[1:]:
                    counter[0] += 1
                    out.append({
                        "debug": inst.get("debug", 0),
                        "engine": inst["engine"],
                        "ins": [], "outs": [],
                        "name": f"synthmmw-{counter[0]}",
                        "opcode": "EventSemaphore",
                        "sync_info": {"on_update": [], "on_wait": [w]},
                    })
                inst["sync_info"] = {"on_update": ou, "on_wait": ow[:1]}
                out.append(inst)
                continue
            if inst.get("opcode") == "Drain" and (len(ow) > 1 or len(ou) > 1):
                for w in ow[1:]:
                    counter[0] += 1
                    out.append({
                        "debug": inst.get("debug", 0),
                        "engine": inst["engine"],
                        "ins": [], "outs": [],
                        "name": f"synthwait-{counter[0]}",
                        "opcode": "EventSemaphore",
                        "sync_info": {"on_update": [], "on_wait": [w]},
                    })
                inst["sync_info"] = {"on_update": ou[:1], "on_wait": ow[:1]}
                out.append(inst)
                for u in ou[1:]:
                    counter[0] += 1
                    out.append({
                        "debug": inst.get("debug", 0),
                        "engine": inst["engine"],
                        "ins": [], "outs": [],
                        "name": f"synthupd-{counter[0]}",
                        "opcode": "EventSemaphore",
                        "sync_info": {"on_update": [u], "on_wait": []},
                    })
            else:
                out.append(inst)
        b["instructions"] = out
        for sb in b.get("blocks", []):
            fix_block(sb)

    for fn in d.get("functions", []):
        for blk in fn.get("blocks", []):
            fix_block(blk)
    return json.dumps(d).encode()


def _install_drainfix():
    import concourse.bass as bass
    if getattr(bass.Bass, "_drainfix_installed", False):
        return
    orig = bass.Bass.to_json_bytes

    def patched(self):
        return _fix_module_json(orig(self))

    bass.Bass.to_json_bytes = patched
    bass.Bass._drainfix_installed = True


# ---------------------------------------------------------------- program
def _build_nc(reps=1):
    import concourse.bass as bass
    import concourse.mybir as mybir
    from concourse.tile import TileContext
    from contextlib import ExitStack

    f32 = mybir.dt.float32
    f32r = mybir.dt.float32r
    bf16 = mybir.dt.bfloat16
    i16 = mybir.dt.int16
    EXP = mybir.ActivationFunctionType.Exp
    MUL = mybir.AluOpType.mult
    ADD = mybir.AluOpType.add

    nc = bass.Bass("TRN2", num_devices=NCORES)

    xqT = nc.dram_tensor("xqT", [D, S], bf16, kind="ExternalInput")
    xkT = nc.dram_tensor("xkT", [D, S], bf16, kind="ExternalInput")
    xvT = nc.dram_tensor("xvT", [D, S], bf16, kind="ExternalInput")
    wq = nc.dram_tensor("wq", [D, CPC], bf16, kind="ExternalInput")
    wk = nc.dram_tensor("wk", [D, CPC], bf16, kind="ExternalInput")
    wv = nc.dram_tensor("wv", [D, CPC], bf16, kind="ExternalInput")
    bqd = nc.dram_tensor("bq", [CPC], f32, kind="ExternalInput")
    bkd = nc.dram_tensor("bk", [CPC], f32, kind="ExternalInput")
    bvd = nc.dram_tensor("bv", [CPC], f32, kind="ExternalInput")
    outd = nc.dram_tensor("out", [CPC, S], f32, kind="ExternalOutput")
    # scratch for the denominator-reciprocal broadcast bounce (stride-0
    # partition reads are only legal from DRAM)
    recscr = nc.dram_tensor("recscr", [2 * NPH, SQC], f32, kind="Internal")

    with ExitStack() as ctx:
        ctx.enter_context(nc.allow_low_precision(
            reason="qk in bf16 + bit-trick exp; matmul accumulates f32; "
                   "rel-err gate 2e-2"))
        tc = ctx.enter_context(TileContext(nc))
        sb = ctx.enter_context(tc.tile_pool(name="sb", bufs=1))
        ps = ctx.enter_context(tc.tile_pool(name="ps", bufs=1, space="PSUM"))

        # ---- resident weights / constants (wk + xk first: critical) ----
        # per-dd W tiles: the first projection MM waits only on dd=0's DMA
        wk_sb = [sb.tile([128, CPC], bf16, name=f"wk_sb{dd}")
                 for dd in range(NDT)]
        for dd in range(NDT):
            nc.sync.dma_start(out=wk_sb[dd],
                              in_=wk[dd * 128:(dd + 1) * 128, :])
        bk_sb = sb.tile([128, NP], f32, name="bk_sb")
        nc.sync.dma_start(out=bk_sb, in_=bkd.rearrange("(p c) -> c p", p=NP))
        bq_sb = sb.tile([128, NP], f32, name="bq_sb")
        nc.sync.dma_start(out=bq_sb, in_=bqd.rearrange("(p c) -> c p", p=NP))

        # persistent activation tiles
        qT = [sb.tile([128, S], bf16, name=f"qT{p}") for p in range(NP)]
        kT = [sb.tile([128, S], bf16, name=f"kT{p}") for p in range(NP)]
        v_aug = sb.tile([128, NSK, 8, 65], bf16, name="v_aug")

        # ---- helpers ----
        def load_x2(x_dram, u):
            """8 [128, 1024] bf16 tiles covering t-chunks 2u, 2u+1 (2KB
            per-partition DMA lines)."""
            xts = []
            for dd in range(NDT):
                x_t = sb.tile([128, 2 * SQC], bf16, tag="xs", bufs=10,
                              name=f"x_{u}_{dd}")
                nc.sync.dma_start(
                    out=x_t,
                    in_=x_dram[dd * 128:(dd + 1) * 128,
                               u * 2 * SQC:(u + 1) * 2 * SQC])
                xts.append(x_t)
            return xts

        def load_x1(x_dram, t):
            """8 single-t [128, 512] bf16 tiles (1KB lines; used for the qT
            passes so the critical path to the first QK is shorter)."""
            xts = []
            for dd in range(NDT):
                x_t = sb.tile([128, SQC], bf16, tag="xs1", bufs=16,
                              name=f"x1_{t}_{dd}")
                nc.sync.dma_start(
                    out=x_t,
                    in_=x_dram[dd * 128:(dd + 1) * 128,
                               t * SQC:(t + 1) * SQC])
                xts.append(x_t)
            return xts

        def emit_proj(t, plist, w_sb, bias_sb, dst, xts, single=False):
            """dst[p][:, t*512:(t+1)*512] = (x @ W + b).T chunk (bf16).
            xts are the [128, 1024] pair tiles for u = t//2 (or single-t
            [128, 512] tiles if single)."""
            xo = 0 if single else (t % 2) * SQC
            for p in plist:
                pr = ps.tile([128, SQC], f32, tag="vp", bufs=2,
                             name=f"prj_{t}_{p}")
                for dd in range(NDT):
                    nc.tensor.matmul(
                        pr[:, :],
                        w_sb[dd][:, p * 128:(p + 1) * 128],
                        xts[dd][:, xo:xo + SQC],
                        start=(dd == 0), stop=(dd == NDT - 1))
                nc.vector.tensor_scalar_add(
                    dst[p][:, t * SQC:(t + 1) * SQC], pr[:, :],
                    bias_sb[:, p:p + 1])

        xv_group = {}  # g -> list of 8 [128, 512] bf16 tiles (tt 4g..4g+3)

        def load_xv(g):
            xvt = []
            for dd in range(NDT):
                xv_t = sb.tile([128, 4 * 128], bf16, tag="xv", bufs=12,
                               name=f"xv_{g}_{dd}")
                nc.sync.dma_start(
                    out=xv_t,
                    in_=xvT[dd * 128:(dd + 1) * 128,
                            g * 512:(g + 1) * 512])
                xvt.append(xv_t)
            xv_group[g] = xvt

        def emit_proj_v(tt):
            """v_aug[:, tt, h, 0:64] = (xv @ Wv + bv) rows tt*128.., bf16."""
            xvt = xv_group[tt // 4]
            co = (tt % 4) * 128
            pv = ps.tile([128, CPC], f32, tag="vp", bufs=2, name=f"pv_{tt}")
            for dd in range(NDT):
                nc.tensor.matmul(
                    pv[:, :], xvt[dd][:, co:co + 128], wv_sb[dd][:, :],
                    start=(dd == 0), stop=(dd == NDT - 1))
            nc.vector.tensor_add(
                v_aug[:, tt, :, 0:64],
                pv.rearrange("c (h d) -> c h d", h=8),
                bv_bc.rearrange("c (h d) -> c h d", h=8))
            if tt % 4 == 3:
                xv_group.pop(tt // 4)

        alpha = {}     # (c, i) -> [128, 2, 512] bf16 pair tile
        av_tiles = {}  # c -> {h: psum tile [65, 512]}

        def emit_qk_i(c, i):
            p, j = divmod(c, NJ)
            scp = ps.tile([128, 2, SQC], f32, tag="sc", bufs=2,
                          name=f"sc_{c}_{i}")
            for h in range(2):
                nc.tensor.matmul(
                    scp[:, h, :],
                    kT[p][h * 64:(h + 1) * 64, i * 128:(i + 1) * 128],
                    qT[p][h * 64:(h + 1) * 64, j * SQC:(j + 1) * SQC],
                    start=True, stop=True)
            a_p = sb.tile([128, 2, SQC], bf16, tag="alpha", bufs=18,
                          name=f"al_{c}_{i}")
            if i in DVE_PAIRS_I:
                nc.vector.tensor_scalar(
                    a_p.bitcast(i16), scp[:, :, :], EXPA, EXPB, MUL, ADD)
            else:
                nc.scalar.activation(a_p[:, :, :], scp[:, :, :], EXP,
                                     scale=0.125)
            alpha[(c, i)] = a_p

        def emit_av_i(c, i):
            p, j = divmod(c, NJ)
            if i == 0:
                av_tiles[c] = {
                    h: ps.tile([65, SQC], f32, tag="av", bufs=2,
                               name=f"av_{c}_{h}")
                    for h in range(2)}
            a_p = alpha[(c, i)]
            for h in range(2):
                nc.tensor.matmul(
                    av_tiles[c][h][:, :],
                    v_aug[:, i, 2 * p + h, :],
                    a_p[:, h, :],
                    start=(i == 0), stop=(i == NSK - 1))
            alpha.pop((c, i))

        norm_state = {}  # c -> (avs, recB)

        def emit_norm_head(c):
            """Free the av PSUM banks FAST: both copies on DVE (whose last
            trick pair is slot 14, so it's free at the boundary). Then the
            denominator bounce: raw row -> DRAM -> [64,8] spread so the
            6-cycles/elem DVE reciprocal costs 8 elems/lane -> DRAM ->
            [64,512] broadcast."""
            avt = av_tiles.pop(c)
            avs, recB = {}, {}
            for h in range(2):
                avs[h] = sb.tile([65, SQC], f32, tag="avs", bufs=4,
                                 name=f"avs_{c}_{h}")
                nc.vector.tensor_copy(avs[h][:, :], avt[h][:, :])
            for h in range(2):
                slot = 2 * c + h
                nc.sync.dma_start(out=recscr[slot:slot + 1, :],
                                  in_=avs[h][64:65, :])
                _r = recscr[slot:slot + 1, :]
                den8 = sb.tile([64, 8], f32, tag="den8", bufs=4,
                               name=f"den8_{c}_{h}")
                nc.sync.dma_start(
                    out=den8,
                    in_=bass.AP(tensor=_r.tensor, offset=_r.offset,
                                ap=[[8, 64], [1, 8]]))
                rec8 = sb.tile([64, 8], f32, tag="rec8", bufs=4,
                               name=f"rec8_{c}_{h}")
                nc.vector.reciprocal(rec8[:, :], den8[:, :])
                nc.sync.dma_start(
                    out=bass.AP(tensor=_r.tensor, offset=_r.offset,
                                ap=[[8, 64], [1, 8]]),
                    in_=rec8[:, :])
                recB[h] = sb.tile([64, SQC], f32, tag="recB", bufs=4,
                                  name=f"recB_{c}_{h}")
                nc.sync.dma_start(
                    out=recB[h],
                    in_=bass.AP(tensor=_r.tensor, offset=_r.offset,
                                ap=[[0, 64]] + list(_r.ap)[1:]))
            norm_state[c] = (avs, recB)

        def _emit_norm_last(c, h):
            """Latency-optimized norm for the final chunk: direct DVE
            reciprocal of the row (no [64,8] spread), ONE DRAM bounce for
            the broadcast, DVE multiply."""
            p, j = divmod(c, NJ)
            avt = av_tiles[c]
            r0 = (2 * p + h) * 64
            slot = 2 * c + h
            avs = sb.tile([65, SQC], f32, tag="avs", bufs=4,
                          name=f"avsL_{h}")
            nc.vector.tensor_copy(avs[:, :], avt[h][:, :])
            rec = sb.tile([1, SQC], f32, tag="recL", bufs=2,
                          name=f"recL_{h}")
            nc.vector.reciprocal(rec[:, :], avs[64:65, :])
            nc.sync.dma_start(out=recscr[slot:slot + 1, :], in_=rec[:, :])
            _r = recscr[slot:slot + 1, :]
            recB = sb.tile([64, SQC], f32, tag="recB", bufs=4,
                           name=f"recBL_{h}")
            nc.sync.dma_start(
                out=recB,
                in_=bass.AP(tensor=_r.tensor, offset=_r.offset,
                            ap=[[0, 64]] + list(_r.ap)[1:]))
            cx = sb.tile([64, SQC], f32, tag="cx", bufs=4,
                         name=f"cxL_{h}")
            nc.vector.tensor_mul(cx[:, :], avs[0:64, :], recB[:, :])
            nc.sync.dma_start(
                out=outd[r0:r0 + 64, j * SQC:(j + 1) * SQC],
                in_=cx[:, :])

        def emit_norm_tail(c):
            """Normalize-multiply on the idle Pool engine (all-SBUF), DMA
            out. Emitted mid-next-phase so the recB bounce has landed."""
            p, j = divmod(c, NJ)
            avs, recB = norm_state.pop(c)
            for h in range(2):
                r0 = (2 * p + h) * 64
                cx = sb.tile([64, SQC], f32, tag="cx", bufs=4,
                             name=f"cx_{c}_{h}")
                nc.gpsimd.tensor_mul(cx[:, :], avs[h][0:64, :], recB[h][:, :])
                nc.sync.dma_start(
                    out=outd[r0:r0 + 64, j * SQC:(j + 1) * SQC],
                    in_=cx[:, :])

        # late-loaded residents (emitted after critical-path DMAs above,
        # but data only needed from mid-prologue onwards)
        wq_sb = [sb.tile([128, CPC], bf16, name=f"wq_sb{dd}")
                 for dd in range(NDT)]
        wv_sb = [sb.tile([128, CPC], bf16, name=f"wv_sb{dd}")
                 for dd in range(NDT)]
        bv_bc = sb.tile([128, CPC], f32, name="bv_bc")

        def _emit_wq():
            for dd in range(NDT):
                nc.sync.dma_start(
                    out=wq_sb[dd],
                    in_=wq[dd * 128:(dd + 1) * 128, :])

        def _emit_wv():
            for dd in range(NDT):
                nc.sync.dma_start(
                    out=wv_sb[dd],
                    in_=wv[dd * 128:(dd + 1) * 128, :])
            _bva = bvd[:]
            nc.sync.dma_start(
                out=bv_bc,
                in_=bass.AP(tensor=_bva.tensor, offset=_bva.offset,
                            ap=[[0, 128]] + list(_bva.ap)))
            nc.gpsimd.memset(v_aug[:, :, :, 64:65], 1.0)

        # ---- emission schedule ----
        def _emit_all():
            # prologue (x/W in bf16, [128,1024] x pair-tiles): kT t0/t1 ->
            # qT t0/t1 -> phase-0 QK 0..7 -> kT t2/t3 -> QK 8..15 ->
            # qT t2/t3 -> wv. wq DMA right after xk u0 so the first qT
            # chain isn't blocked.
            xkt0 = load_x1(xkT, 0)
            _emit_wq()
            emit_proj(0, [0, 1, 2, 3], wk_sb, bk_sb, kT, xkt0, single=True)
            xq0 = load_x1(xqT, 0)
            emit_proj(0, [0, 1, 2, 3], wq_sb, bq_sb, qT, xq0, single=True)
            for i in range(4):
                emit_qk_i(0, i)
            xkt1 = load_x1(xkT, 1)
            emit_proj(1, [0, 1, 2, 3], wk_sb, bk_sb, kT, xkt1, single=True)
            xk1 = load_x2(xkT, 1)
            emit_proj(2, [0, 1, 2, 3], wk_sb, bk_sb, kT, xk1)
            for i in range(4, 8):
                emit_qk_i(0, i)
            emit_proj(3, [0, 1, 2, 3], wk_sb, bk_sb, kT, xk1)
            for i in range(8, 12):
                emit_qk_i(0, i)
            xq1 = load_x1(xqT, 1)
            emit_proj(1, [0, 1, 2, 3], wq_sb, bq_sb, qT, xq1, single=True)
            for i in range(12, 16):
                emit_qk_i(0, i)
            xq2 = load_x1(xqT, 2)
            emit_proj(2, [0, 1, 2, 3], wq_sb, bq_sb, qT, xq2, single=True)
            xq3 = load_x1(xqT, 3)
            emit_proj(3, [0, 1, 2, 3], wq_sb, bq_sb, qT, xq3, single=True)
            _emit_wv()

            # phase 1: v-projection (xv DMA paced) interleaved with QK(1)
            # and AV(0) (AV(0,i) needs v_aug[:, i] just computed).
            for i in range(NSK):
                if i == 0:
                    load_xv(0)
                if i % 4 == 1 and i // 4 + 1 < 4:
                    load_xv(i // 4 + 1)
                emit_proj_v(i)
                emit_qk_i(1, i)
                emit_av_i(0, i)
            emit_norm_head(0)

            # phases 2..15: pure QK/AV; norm tail of phase c-2 mid-phase
            # (after its recB bounce has landed), norm head of c-1 at end.
            # AV MMs grouped by head across i-slot PAIRS so consecutive AV
            # MMs accumulate into the SAME PSUM bank (drain overlaps fill).
            def emit_av_pair(c, ii):
                p, j = divmod(c, NJ)
                if ii == 0:
                    av_tiles[c] = {
                        h: ps.tile([65, SQC], f32, tag="av", bufs=2,
                                   name=f"av_{c}_{h}")
                        for h in range(2)}
                for h in range(2):
                    for i2 in (ii, ii + 1):
                        nc.tensor.matmul(
                            av_tiles[c][h][:, :],
                            v_aug[:, i2, 2 * p + h, :],
                            alpha[(c, i2)][:, h, :],
                            start=(i2 == 0), stop=(i2 == NSK - 1))
                alpha.pop((c, ii))
                alpha.pop((c, ii + 1))

            for c in range(2, NPH):
                for ii in range(0, NSK, 2):
                    emit_qk_i(c, ii)
                    emit_qk_i(c, ii + 1)
                    emit_av_pair(c - 1, ii)
                    if ii == 6:
                        emit_norm_tail(c - 2)
                emit_norm_head(c - 1)

            # epilogue: run h0's AV chain first so norm(15) h0 (copy +
            # direct reciprocal + single bounce + DVE mul) overlaps h1's
            # chain; latency-optimized norm path for the final chunk.
            c = NPH - 1
            p, j = divmod(c, NJ)
            av_tiles[c] = {
                h: ps.tile([65, SQC], f32, tag="av", bufs=2,
                           name=f"av_{c}_{h}")
                for h in range(2)}
            for h in range(2):
                for i in range(NSK):
                    nc.tensor.matmul(
                        av_tiles[c][h][:, :],
                        v_aug[:, i, 2 * p + h, :],
                        alpha[(c, i)][:, h, :],
                        start=(i == 0), stop=(i == NSK - 1))
                if h == 0:
                    emit_norm_tail(NPH - 2)
                    _emit_norm_last(c, 0)
            for i in range(NSK):
                alpha.pop((c, i))
            _emit_norm_last(c, 1)

        for _rep in range(reps):
            _emit_all()

    return nc


_NC_BY_REPS = {}


def _get_nc(reps=1):
    if reps not in _NC_BY_REPS:
        _install_drainfix()
        _NC_BY_REPS[reps] = _build_nc(reps)
    return _NC_BY_REPS[reps]


# ---------------------------------------------------------------- entry
def build_in_maps(inputs):
    import ml_dtypes

    bf16 = ml_dtypes.bfloat16
    query = np.asarray(inputs["query"], np.float32)
    key_in = np.asarray(inputs["key_in"], np.float32)
    value = np.asarray(inputs["value"], np.float32)
    Wq = np.asarray(inputs["Wq"], np.float32)
    Wk = np.asarray(inputs["Wk"], np.float32)
    Wv = np.asarray(inputs["Wv"], np.float32)
    bq = np.asarray(inputs["bq"], np.float32)
    bk = np.asarray(inputs["bk"], np.float32)
    bv = np.asarray(inputs["bv"], np.float32)

    in_maps = []
    for c in range(NCORES):
        b, hg = divmod(c, 2)
        cols = slice(hg * CPC, (hg + 1) * CPC)
        in_maps.append({
            "xqT": np.ascontiguousarray(query[b].T.astype(bf16)),
            "xkT": np.ascontiguousarray(key_in[b].T.astype(bf16)),
            "xvT": np.ascontiguousarray(value[b].T.astype(bf16)),
            "wq": np.ascontiguousarray(Wq[:, cols].astype(bf16)),
            "wk": np.ascontiguousarray(Wk[:, cols].astype(bf16)),
            "wv": np.ascontiguousarray(Wv[:, cols].astype(bf16)),
            "bq": np.ascontiguousarray(bq[cols]),
            "bk": np.ascontiguousarray(bk[cols]),
            "bv": np.ascontiguousarray(bv[cols]),
        })
    return in_maps


def kernel(query, key_in, value, Wq, bq, Wk, bk, Wv, bv):
    from concourse.bass_utils import run_bass_kernel_spmd

    nc = _get_nc()
    in_maps = build_in_maps({
        "query": query, "key_in": key_in, "value": value,
        "Wq": Wq, "bq": bq, "Wk": Wk, "bk": bk, "Wv": Wv, "bv": bv,
    })

    res = run_bass_kernel_spmd(nc, in_maps, core_ids=list(range(NCORES)))

    out = np.empty((B, S, D), np.float32)
    for c in range(NCORES):
        b, hg = divmod(c, 2)
        out[b, :, hg * CPC:(hg + 1) * CPC] = res.results[c]["out"].T
    return out
